# revision 1
# baseline (speedup 1.0000x reference)
"""Causal self-attention block (global-RMSNorm + MHA + SwiGLU) on 8 TRN2 NeuronCores.

Sharding: data-parallel over batch (8 batch elements -> 8 cores), weights
replicated.  The two RMSNorms take a global mean over ALL elements, so each
core AllGathers its 128 per-partition partial sums of squares; the 1/rms
scalar is applied to PSUM outputs after the (x*g)@W matmuls, which lets the
collectives overlap with the heavy matmuls.

Layout: activations kept transposed [feature, token].  Attention computes
S^T = K.Q^T (k on partitions) so softmax exp is a single ACT pass
PSUM->SBUF with no transposes; the softmax denominator falls out of the
P@V matmul via an appended ones-column on V.  Matmul inputs are bf16
(full PE rate); accumulation, softmax and the residual stream stay fp32.
"""

import math

import numpy as np

T = 1024  # tokens per batch element
E = 512  # embedding
H = 8  # heads
S = 64  # head dim
DFF = 1365
NDC = 11  # ceil(DFF/128) dff chunks (last padded)
DFFP = NDC * 128
EPS = 1e-5
SCALE = 1.0 / math.sqrt(E)
NCORES = 8
NTOT = float(NCORES * T * E)  # global element count for the RMS mean

TC = T // 128  # 8 token chunks
EC = E // 128  # 4 embedding chunks
NEG = -1.0e30


def build_nc(sim_safe=False, loop_reps=0, qkv_fast=True):
    """loop_reps>0 builds a timing variant: the whole body runs inside a
    hardware For_i loop with the collectives replaced by a local DMA
    roundtrip of the same shape (collectives cannot sit in control flow).
    The graded path is loop_reps=0."""
    import concourse.bass as bass  # noqa: F401
    import concourse.mybir as mybir
    from concourse import bacc
    from concourse.masks import make_identity
    from concourse.tile import TileContext

    f32 = mybir.dt.float32
    bf16 = mybir.dt.bfloat16
    mult = mybir.AluOpType.mult
    add = mybir.AluOpType.add

    nc = bacc.Bacc(None, target_bir_lowering=False, num_devices=NCORES)

    x_d = nc.dram_tensor("x_b", [T, E], f32, kind="ExternalInput")
    Ww_d = nc.dram_tensor("W_w", [E, 3 * E], f32, kind="ExternalInput")
    Wb_d = nc.dram_tensor("W_b", [3 * E], f32, kind="ExternalInput")
    Wo_d = nc.dram_tensor("Wo_w", [E, E], f32, kind="ExternalInput")
    Wob_d = nc.dram_tensor("Wo_b", [E], f32, kind="ExternalInput")
    W1_d = nc.dram_tensor("W1_w", [E, DFF], f32, kind="ExternalInput")
    W1b_d = nc.dram_tensor("W1_b", [DFF], f32, kind="ExternalInput")
    W2_d = nc.dram_tensor("W2_w", [DFF, E], f32, kind="ExternalInput")
    W2b_d = nc.dram_tensor("W2_b", [E], f32, kind="ExternalInput")
    W3_d = nc.dram_tensor("W3_w", [E, DFF], f32, kind="ExternalInput")
    W3b_d = nc.dram_tensor("W3_b", [DFF], f32, kind="ExternalInput")
    gm_d = nc.dram_tensor("g_mha", [E], f32, kind="ExternalInput")
    gf_d = nc.dram_tensor("g_ff", [E], f32, kind="ExternalInput")
    out_d = nc.dram_tensor("out", [T, E], f32, kind="ExternalOutput")

    # collective bounce buffers (per-partition partial sums of squares)
    cc1_in = nc.dram_tensor("cc1_in", [128], f32)
    cc1_out = nc.dram_tensor("cc1_out", [NCORES * 128], f32, addr_space="Shared")
    cc2_in = nc.dram_tensor("cc2_in", [128], f32)
    cc2_out = nc.dram_tensor("cc2_out", [NCORES * 128], f32, addr_space="Shared")
    rgroups = [[i for i in range(NCORES)]]

    def _emit(tc, no_cc):
        pass_qkv_fast = qkv_fast
        with (
            tc.tile_pool(name="pp", bufs=1) as pp,
            tc.tile_pool(name="sm", bufs=2) as sm,
            tc.tile_pool(name="psA", bufs=2, space="PSUM") as psA,
            tc.tile_pool(name="psB", bufs=2, space="PSUM") as psB,
        ):
            # ---------- x first: everything downstream gates on it ----------
            xy1 = pp.tile([128, TC, E], f32)
            x_r = x_d.rearrange("(tc p) e -> p tc e", p=128)
            for i in range(0, TC, 2):
                nc.sync.dma_start(xy1[:, i:i + 2, :], x_r[:, i:i + 2, :])
            gm = pp.tile([128, EC], f32)
            nc.sync.dma_start(gm, gm_d.rearrange("(c p) -> p c", p=128))
            gf = pp.tile([128, EC], f32)
            nc.sync.dma_start(gf, gf_d.rearrange("(c p) -> p c", p=128))
            # ---------- persistent constants ----------
            id128 = pp.tile([128, 128], f32)
            make_identity(nc, id128)
            id64b = pp.tile([128, 64], bf16)
            make_identity(nc, id64b[0:64, :])
            make_identity(nc, id64b[64:128, :])
            if not qkv_fast:
                wob_row = pp.tile([1, E], f32)
                nc.sync.dma_start(wob_row, Wob_d[None, :])
                wob_b = pp.tile([128, E], f32)
                nc.gpsimd.partition_broadcast(wob_b, wob_row)
                w2b_row = pp.tile([1, E], f32)
                nc.sync.dma_start(w2b_row, W2b_d[None, :])
                w2b_b = pp.tile([128, E], f32)
                nc.gpsimd.partition_broadcast(w2b_b, w2b_row)
                # qkv biases, permuted to match qT/kT/vT row layout
                qb = pp.tile([128, EC], f32)
                kb = pp.tile([128, EC], f32)
                vb = pp.tile([128, EC], f32)
                for h in range(H):
                    p0 = 64 * (h % 2)
                    ch = h // 2
                    for off, dst in ((0, qb), (64, kb), (128, vb)):
                        nc.sync.dma_start(
                            dst[p0:p0 + 64, ch:ch + 1],
                            Wb_d[192 * h + off:192 * h + off + 64][:, None],
                        )
            else:
                qb = kb = vb = None
            epsT = pp.tile([1, 1], f32)
            nc.vector.memset(epsT, EPS)

            # ---------- ssq(x) -> AllGather #1 ----------
            sq1 = pp.tile([128, TC // 2], f32)
            for i in range(TC // 2):
                scr = sm.tile([128, 2, E], bf16, tag="scrb", name="scr")
                nc.scalar.activation(
                    scr, xy1[:, 2 * i:2 * i + 2, :],
                    mybir.ActivationFunctionType.Square,
                    accum_out=sq1[:, i:i + 1],
                )
            sqc1 = pp.tile([128, 1], f32)
            nc.vector.reduce_sum(sqc1, sq1, axis=mybir.AxisListType.X)
            nc.sync.dma_start(cc1_in[:], sqc1)
            if no_cc:
                rs1 = pp.tile([1, 128], f32, name="rs1")
                nc.sync.dma_start(rs1, cc1_in[None, :])
                sc1 = float(NCORES) / NTOT
            else:
                nc.gpsimd.collective_compute(
                    "AllGather", mybir.AluOpType.bypass, replica_groups=rgroups,
                    ins=[cc1_in[:]], outs=[cc1_out[:]],
                )
                rs1 = pp.tile([1, NCORES * 128], f32)
                nc.sync.dma_start(rs1, cc1_out[None, :])
                sc1 = 1.0 / NTOT
            tot1 = pp.tile([1, 1], f32)
            nc.vector.reduce_sum(tot1, rs1, axis=mybir.AxisListType.X)
            rms1 = pp.tile([1, 1], f32)
            nc.scalar.activation(
                rms1, tot1, mybir.ActivationFunctionType.Sqrt,
                bias=epsT[0:1, 0:1], scale=sc1,
            )
            inv1 = pp.tile([1, 1], f32)
            nc.vector.reciprocal(inv1, rms1)
            if not qkv_fast:
                inv1b = pp.tile([128, 1], f32)
                nc.gpsimd.partition_broadcast(inv1b, inv1)
            else:
                inv1b = None
            if qkv_fast:
                se1 = pp.tile([1, 1], f32)
                nc.vector.tensor_scalar(
                    se1, inv1, inv1[0:1, 0:1], SCALE, mult, mult)
                se_b = pp.tile([128, 1], f32)
                nc.gpsimd.partition_broadcast(se_b, se1)
            else:
                se_b = None

            with tc.tile_pool(name="pC", bufs=1) as pC:
              qT = pC.tile([128, EC, T], bf16)
              kT = pC.tile([128, EC, T], bf16)
              with tc.tile_pool(name="pA", bufs=1) as pA:
                vTb = pA.tile([128, EC, T], bf16)
                # ---------- load QKV weights (permuted per head, cast bf16) --
                ws = pA.tile([128, EC, 3 * E], f32)
                wwr_f = Ww_d.rearrange("(c p) n -> p c n", p=128)
                for h in range(H):
                    nc.sync.dma_start(
                        ws[:, :, 192 * h:192 * h + 192],
                        wwr_f[:, :, 192 * h:192 * h + 192],
                    )
                wq = pA.tile([128, EC, E], bf16)
                wk = pA.tile([128, EC, E], bf16)
                wv = pA.tile([128, EC, E], bf16)
                for h in range(H):
                    for off, dst in ((0, wq), (64, wk), (128, wv)):
                        nc.gpsimd.tensor_copy(
                            out=dst[:, :, 64 * h:64 * h + 64],
                            in_=ws[:, :, 192 * h + off:192 * h + off + 64],
                        )

                # ---------- transpose x, fuse g_mha, cast bf16 ----------
                xgT = pA.tile([128, EC, T], bf16)
                for ec in range(EC):
                    for i in range(TC):
                        pst = psA.tile([128, 128], f32, tag="tr")
                        nc.tensor.transpose(
                            pst, xy1[:, i, 128 * ec:128 * ec + 128], id128)
                        nc.vector.tensor_scalar(
                            xgT[:, ec, 128 * i:128 * i + 128], pst,
                            gm[:, ec:ec + 1], None, mult,
                        )

                # ---------- QKV matmuls (deferred 1/rms via inv1b) ----------
                for w, b, dstT in ((wq, qb, qT), (wk, kb, kT), (wv, vb, vTb)):
                    for c in range(EC):
                        for t2 in range(2):
                            ps = psB.tile([128, 512], f32, tag="mm")
                            for ko in range(EC):
                                nc.tensor.matmul(
                                    ps,
                                    w[:, ko, 128 * c:128 * c + 128],
                                    xgT[:, ko, 512 * t2:512 * t2 + 512],
                                    start=(ko == 0), stop=(ko == EC - 1),
                                )
                            if qkv_fast:
                                nc.vector.tensor_copy(
                                    out=dstT[:, c, 512 * t2:512 * t2 + 512],
                                    in_=ps)
                            else:
                                nc.vector.tensor_scalar(
                                    dstT[:, c, 512 * t2:512 * t2 + 512], ps,
                                    inv1b[:, 0:1], b[:, c:c + 1], mult, add,
                                )

                # ---------- v transpose -> vv[k-part, kc, h, 65] (ones col) ----
                vv = pC.tile([128, TC, H, S + 1], bf16)
                if qkv_fast:
                    # denom column = rms1: divides P.v_raw by D/inv1 = D*rms1
                    bc1 = pp.tile([1, TC * H], bf16)
                    nc.vector.memset(bc1, 1.0)
                    nc.vector.tensor_scalar_mul(bc1, bc1, rms1[0:1, 0:1])
                    bc1h = pp.tile([128, TC * H], bf16)
                    nc.gpsimd.partition_broadcast(bc1h, bc1)
                    nc.vector.tensor_copy(
                        out=vv[:, :, :, S:S + 1].rearrange("p a b c -> p (a b c)"),
                        in_=bc1h)
                else:
                    nc.vector.memset(vv[:, :, :, S:S + 1], 1.0)
                for h in range(H):
                    p0 = 64 * (h % 2)
                    ch = h // 2
                    for kc in range(TC):
                        pst = psA.tile([128, 128], bf16, tag="tr", name="pstv")[:, 0:64]
                        nc.tensor.transpose(
                            pst, vTb[p0:p0 + 64, ch, 128 * kc:128 * kc + 128],
                            id64b[p0:p0 + 64, :],
                        )
                        nc.vector.tensor_copy(out=vv[:, kc, h, 0:S], in_=pst)

              if True:
                yT = pC.tile([128, EC, T], bf16)
                Wo_s = pC.tile([128, EC, E], bf16)
                wor = Wo_d.rearrange("(c p) n -> p c n", p=128)
                for c in range(EC):
                    stgo = sm.tile([128, E], f32, tag="wstg2", name="stgo")
                    nc.sync.dma_start(stgo, wor[:, c, :])
                    nc.gpsimd.tensor_copy(out=Wo_s[:, c, :], in_=stgo)

                # FFN weights: DMA fp32 staging -> bf16 casts (gpsimd), overlapped
                with tc.tile_pool(name="pD", bufs=1) as pD:
                    w1b16 = pD.tile([128, EC, DFFP], bf16)
                    w3b16 = pD.tile([128, EC, DFFP], bf16)
                    w2b16 = pD.tile([128, NDC, E], bf16)
                    nc.vector.memset(w1b16[:, :, DFF:], 0.0)
                    nc.vector.memset(w3b16[:, :, DFF:], 0.0)
                    nc.vector.memset(w2b16[:, NDC - 1, :], 0.0)
                    if not qkv_fast:
                        b1 = pD.tile([128, NDC], f32)
                        b3 = pD.tile([128, NDC], f32)
                        nc.vector.memset(b1[:, NDC - 1:NDC], 0.0)
                        nc.vector.memset(b3[:, NDC - 1:NDC], 0.0)
                    for wd, wt in ((W1_d, w1b16), (W3_d, w3b16)):
                        wr = wd.rearrange("(c p) n -> p c n", p=128)
                        for c in range(EC):
                            for j in range(0, DFF, 512):
                                n = min(512, DFF - j)
                                stg = sm.tile([128, 512], f32, tag="wstg2",
                                              name="stg")
                                nc.sync.dma_start(stg[:, 0:n], wr[:, c, j:j + n])
                                nc.gpsimd.tensor_copy(
                                    out=wt[:, c, j:j + n], in_=stg[:, 0:n])
                    w2r = W2_d[0:1280].rearrange("(c p) n -> p c n", p=128)
                    for c in range(NDC - 1):
                        stg2 = sm.tile([128, E], f32, tag="wstg2", name="stg2")
                        nc.sync.dma_start(stg2, w2r[:, c, :])
                        nc.gpsimd.tensor_copy(out=w2b16[:, c, :], in_=stg2)
                    stg2 = sm.tile([128, E], f32, tag="wstg2", name="stg2")
                    nc.sync.dma_start(stg2[0:85, :], W2_d[1280:DFF, :])
                    nc.gpsimd.tensor_copy(out=w2b16[0:85, NDC - 1, :], in_=stg2[0:85, :])
                    if not qkv_fast:
                        nc.sync.dma_start(
                            b1[:, 0:NDC - 1],
                            W1b_d[0:1280].rearrange("(c p) -> p c", p=128),
                        )
                        nc.sync.dma_start(
                            b1[0:85, NDC - 1:NDC], W1b_d[1280:DFF][:, None])
                        nc.sync.dma_start(
                            b3[:, 0:NDC - 1],
                            W3b_d[0:1280].rearrange("(c p) -> p c", p=128),
                        )
                        nc.sync.dma_start(
                            b3[0:85, NDC - 1:NDC], W3b_d[1280:DFF][:, None])

                    # ---------- attention + Wo (per 512-token q slice) ------
                    # first fold Wo_b into the residual stream
                    if not qkv_fast:
                        for i in range(TC):
                            nc.vector.tensor_tensor(
                                xy1[:, i, :], xy1[:, i, :], wob_b, add)
                    sq2 = pp.tile([128, TC], f32)

                    with (
                        tc.tile_pool(name="pt", bufs=3) as ptp,
                        tc.tile_pool(name="psS", bufs=2, space="PSUM") as psS,
                        tc.tile_pool(name="psV", bufs=2, space="PSUM") as psV,
                    ):
                        for qs in range(2):
                            for ch in range(H // 2):
                                # head pair (2ch, 2ch+1): base partitions 0/64
                                # -> PE row groups run the pair concurrently
                                PTs = [
                                    ptp.tile([128, TC, 512], bf16, tag="PT",
                                             name=f"PT{par}")
                                    for par in range(2)
                                ]
                                nkc = 4 * qs + 4
                                for kc in range(nkc):
                                    q0 = max(512 * qs, 128 * kc)
                                    n = 512 * qs + 512 - q0
                                    q0l = q0 - 512 * qs
                                    for par in range(2):
                                        p0 = 64 * par
                                        PT = PTs[par]
                                        ps = psS.tile([128, 512], f32, tag="sc")
                                        nc.tensor.matmul(
                                            ps[:, 0:n],
                                            kT[p0:p0 + 64, ch,
                                               128 * kc:128 * kc + 128],
                                            qT[p0:p0 + 64, ch, q0:q0 + n],
                                            start=True, stop=True,
                                        )
                                        nc.scalar.activation(
                                            PT[:, kc, q0l:q0l + n],
                                            ps[:, 0:n],
                                            mybir.ActivationFunctionType.Exp,
                                            scale=(se_b[:, 0:1] if qkv_fast
                                                   else SCALE),
                                        )
                                        if kc >= 4 * qs:
                                            d0 = 128 * kc - 512 * qs
                                            nc.gpsimd.affine_select(
                                                out=PT[:, kc, d0:d0 + 128],
                                                in_=PT[:, kc, d0:d0 + 128],
                                                compare_op=mybir.AluOpType.is_ge,
                                                fill=0.0, base=0,
                                                pattern=[[1, 128]],
                                                channel_multiplier=-1,
                                            )
                                psys = [
                                    psV.tile([128, 512], f32, tag="pv",
                                             name=f"psy{par}")
                                    for par in range(2)
                                ]
                                for kc in range(nkc):
                                    off = max(0, 128 * kc - 512 * qs)
                                    for par in range(2):
                                        nc.tensor.matmul(
                                            psys[par][0:S + 1, off:512],
                                            vv[:, kc, 2 * ch + par, :],
                                            PTs[par][:, kc, off:512],
                                            start=(kc == 0),
                                            stop=(kc == nkc - 1),
                                            skip_group_check=True,
                                        )
                                for par in range(2):
                                    p0 = 64 * par
                                    psy = psys[par]
                                    rd = sm.tile([1, 512], f32, tag="rd")
                                    nc.vector.reciprocal(rd, psy[S:S + 1, :])
                                    rdb = sm.tile([128, 512], f32, tag="rdb")
                                    nc.gpsimd.partition_broadcast(rdb, rd)
                                    nc.vector.tensor_tensor(
                                        yT[p0:p0 + 64, ch,
                                           512 * qs:512 * qs + 512],
                                        psy[0:S, :], rdb[0:S, :], mult,
                                    )
                            # Wo + residual for this q slice
                            for qc in range(4 * qs, 4 * qs + 4):
                                ps = psB.tile([128, 512], f32, tag="mm")
                                for ko in range(EC):
                                    nc.tensor.matmul(
                                        ps,
                                        yT[:, ko, 128 * qc:128 * qc + 128],
                                        Wo_s[:, ko, :],
                                        start=(ko == 0), stop=(ko == EC - 1),
                                    )
                                nc.vector.tensor_tensor(
                                    xy1[:, qc, :], ps, xy1[:, qc, :], add)
                                scr2 = sm.tile([128, E], bf16, tag="scrb",
                                               name="scr2")
                                nc.scalar.activation(
                                    scr2, xy1[:, qc, :],
                                    mybir.ActivationFunctionType.Square,
                                    accum_out=sq2[:, qc:qc + 1],
                                )
                    # ---------- AllGather #2 ----------
                    sqc2 = pp.tile([128, 1], f32)
                    nc.vector.reduce_sum(sqc2, sq2, axis=mybir.AxisListType.X)
                    nc.sync.dma_start(cc2_in[:], sqc2)
                    if no_cc:
                        rs2 = pp.tile([1, 128], f32, name="rs2")
                        nc.sync.dma_start(rs2, cc2_in[None, :])
                        sc2 = float(NCORES) / NTOT
                    else:
                        nc.gpsimd.collective_compute(
                            "AllGather", mybir.AluOpType.bypass,
                            replica_groups=rgroups,
                            ins=[cc2_in[:]], outs=[cc2_out[:]],
                        )
                        rs2 = pp.tile([1, NCORES * 128], f32)
                        nc.sync.dma_start(rs2, cc2_out[None, :])
                        sc2 = 1.0 / NTOT
                    tot2 = pp.tile([1, 1], f32)
                    nc.vector.reduce_sum(tot2, rs2, axis=mybir.AxisListType.X)
                    rms2 = pp.tile([1, 1], f32)
                    nc.scalar.activation(
                        rms2, tot2, mybir.ActivationFunctionType.Sqrt,
                        bias=epsT[0:1, 0:1], scale=sc2,
                    )
                    inv2 = pp.tile([1, 1], f32)
                    nc.vector.reciprocal(inv2, rms2)
                    inv2b = pp.tile([128, 1], f32)
                    nc.gpsimd.partition_broadcast(inv2b, inv2)

                    # ---------- transpose y1, fuse g_ff ----------
                    y1gT = pD.tile([128, EC, T], bf16)
                    for ec in range(EC):
                        for i in range(TC):
                            pst = psA.tile([128, 128], f32, tag="tr")
                            nc.tensor.transpose(
                                pst, xy1[:, i, 128 * ec:128 * ec + 128], id128)
                            nc.vector.tensor_scalar(
                                y1gT[:, ec, 128 * i:128 * i + 128], pst,
                                gf[:, ec:ec + 1], None, mult,
                            )
                    # fold W2_b into residual stream (after transposes read y1)
                    if not qkv_fast:
                        for i in range(TC):
                            nc.vector.tensor_tensor(
                                xy1[:, i, :], xy1[:, i, :], w2b_b, add)

                    # ---------- SwiGLU FFN ----------
                    hT = pD.tile([128, NDC, T], bf16)
                    z1s = pD.tile([128, 8, 512], bf16)
                    z3s = pD.tile([128, 8, 512], bf16)
                    with (
                        tc.tile_pool(name="psF1", bufs=2, space="PSUM") as psF1,
                        tc.tile_pool(name="psF3", bufs=2, space="PSUM") as psF3,
                    ):
                        for qs in range(2):
                            for dc in range(NDC):
                                ps1 = psF1.tile([128, 512], f32, tag="f1")
                                ps3 = psF3.tile([128, 512], f32, tag="f3")
                                for ko in range(EC):
                                    nc.tensor.matmul(
                                        ps1,
                                        w1b16[:, ko, 128 * dc:128 * dc + 128],
                                        y1gT[:, ko, 512 * qs:512 * qs + 512],
                                        start=(ko == 0), stop=(ko == EC - 1),
                                    )
                                for ko in range(EC):
                                    nc.tensor.matmul(
                                        ps3,
                                        w3b16[:, ko, 128 * dc:128 * dc + 128],
                                        y1gT[:, ko, 512 * qs:512 * qs + 512],
                                        start=(ko == 0), stop=(ko == EC - 1),
                                    )
                                h1s = sm.tile([128, 512], f32, tag="h1s")
                                bb1 = 0.0 if qkv_fast else b1[:, dc:dc + 1]
                                bb3 = 0.0 if qkv_fast else b3[:, dc:dc + 1]
                                if qs == 0 and dc < 8:
                                    # free PSUM early: AllGather #2 runway
                                    nc.vector.tensor_copy(
                                        out=z1s[:, dc, :], in_=ps1)
                                    nc.vector.tensor_copy(
                                        out=z3s[:, dc, :], in_=ps3)
                                    ps1 = z1s[:, dc, :]
                                    ps3 = z3s[:, dc, :]
                                if sim_safe:
                                    # CoreSim lacks Silu; exact x*sigmoid(x)
                                    nc.scalar.activation(
                                        h1s, ps1,
                                        mybir.ActivationFunctionType.Sigmoid,
                                        bias=bb1, scale=inv2b[:, 0:1],
                                    )
                                    h1l = sm.tile([128, 512], f32, tag="h1l",
                                                  bufs=1)
                                    nc.vector.tensor_scalar(
                                        h1l, ps1, inv2b[:, 0:1], bb1,
                                        mult, add,
                                    )
                                    nc.vector.tensor_tensor(h1s, h1s, h1l, mult)
                                else:
                                    nc.scalar.activation(
                                        h1s, ps1, mybir.ActivationFunctionType.Silu,
                                        bias=bb1, scale=inv2b[:, 0:1],
                                    )
                                h3s = sm.tile([128, 512], f32, tag="h3s")
                                nc.scalar.activation(
                                    h3s, ps3,
                                    mybir.ActivationFunctionType.Identity,
                                    bias=bb3, scale=inv2b[:, 0:1],
                                )
                                nc.vector.tensor_tensor(
                                    hT[:, dc, 512 * qs:512 * qs + 512],
                                    h1s, h3s, mult,
                                )
                            for qc in range(4 * qs, 4 * qs + 4):
                                ps = psB.tile([128, 512], f32, tag="mm")
                                for dc in range(NDC):
                                    nc.tensor.matmul(
                                        ps,
                                        hT[:, dc, 128 * qc:128 * qc + 128],
                                        w2b16[:, dc, :],
                                        start=(dc == 0), stop=(dc == NDC - 1),
                                    )
                                ot = sm.tile([128, 512], f32, tag="ot")
                                nc.vector.tensor_tensor(
                                    ot, ps, xy1[:, qc, :], add)
                                nc.sync.dma_start(
                                    out_d[128 * qc:128 * qc + 128, :], ot)

    with TileContext(nc) as tc:
        if loop_reps > 0:
            with tc.For_i(0, loop_reps, 1):
                _emit(tc, no_cc=True)
        else:
            _emit(tc, no_cc=False)

    nc.finalize()
    return nc


_W_NAMES = [
    "W_w", "W_b", "Wo_w", "Wo_b", "W1_w", "W1_b", "W2_w", "W2_b",
    "W3_w", "W3_b", "g_mha", "g_ff",
]


def kernel(**inputs) -> np.ndarray:
    from concourse.bass_utils import run_bass_kernel_spmd

    fast = all(
        not np.any(np.asarray(inputs[k]))
        for k in ("W_b", "Wo_b", "W1_b", "W2_b", "W3_b")
    )
    nc = build_nc(qkv_fast=fast)
    x = np.ascontiguousarray(np.asarray(inputs["x"], dtype=np.float32))
    base = {
        k: np.ascontiguousarray(np.asarray(inputs[k], dtype=np.float32))
        for k in _W_NAMES
    }
    in_maps = [dict(base, x_b=np.ascontiguousarray(x[c])) for c in range(NCORES)]
    res = run_bass_kernel_spmd(nc, in_maps, core_ids=list(range(NCORES)))
    return np.stack([r["out"] for r in res.results], axis=0).astype(np.float32)

